# revision 17
# baseline (speedup 1.0000x reference)
"""Trainium2 Bass kernel for CycleEnergyDirectAddNoHead (retrieval soft-kNN).

Math per pair p (8 pairs, one per NeuronCore):
    dist  = (||u||^2 + ||v||^2 - 2 u@v^T) * COEF        [U, V]
    score = softmax(dist, axis=v)
    neg   = score @ v                                    [U, F]
    out   = concat([u.reshape(-1, F), neg.reshape(-1, F)], 0)

Device-side simplifications:
  * ||u||^2 is constant along the softmax axis -> dropped (softmax invariant).
  * u is pre-scaled by -2*COEF on the host during dtype conversion, so the
    PE accumulates -2*COEF*uv directly; COEF*vv is added into the same PSUM
    via a K=1 broadcast matmul with a ones vector.
  * The big F-contraction matmul runs in split-bf16 (hi/lo) for near-f32
    accuracy at bf16 PE rates.  Host feeds pre-transposed, partition-tiled
    layouts so every DMA is contiguous.
  * softmax normalization is folded into the PSUM->SBUF copyback of the
    second matmul (per-partition 1/sum scale).
"""

import os
import threading

import numpy as np
import ml_dtypes

BF16 = ml_dtypes.bfloat16
COEF = 0.1
N_CORES = 8
U = 256
V = 256
F = 12544
KT = F // 128          # 98 k-tiles for the F-contraction
NTERM = int(os.environ.get("ATHENA_NTERM", "3"))   # split-bf16 terms in uv matmul
OUT_BF16 = bool(int(os.environ.get("ATHENA_OUT_BF16", "0")))
KCH = 7                # k-tiles per mm1 feed DMA chunk (98 = 14*7)
FCH = 1568             # f-chunk for the natural-v stream (12544 = 8*1568)
NCH = 512              # mm2 output chunk (psum bank width)
NG = 4                 # mm2 chunks per output DMA group

WU = 512 if NTERM >= 2 else 256   # packed width of the u^T feed
WV = 512 if NTERM >= 3 else 256   # packed width of the v^T feed

LAST_RESULTS = None    # BassKernelResults of the most recent run (for test.py)

_nc_cache = {}
_nc_lock = threading.Lock()


def _build_nc():
    from contextlib import ExitStack

    import concourse.bass as bass
    import concourse.mybir as mybir
    import concourse.tile as tile
    from concourse.masks import make_identity

    fp32 = mybir.dt.float32
    bf16 = mybir.dt.bfloat16
    out_dt = bf16 if OUT_BF16 else fp32
    AF = mybir.ActivationFunctionType
    X = mybir.AxisListType.X

    nc = bass.Bass()
    ut = nc.dram_tensor("ut", [128, KT, WU], bf16, kind="ExternalInput")
    vt = nc.dram_tensor("vt", [128, KT, WV], bf16, kind="ExternalInput")
    vn = nc.dram_tensor("vn", [2, 128, F], fp32, kind="ExternalInput")
    out = nc.dram_tensor("out", [2, 128, F], out_dt, kind="ExternalOutput")

    with tile.TileContext(nc) as tc, ExitStack() as ctx:
        singles = ctx.enter_context(tc.tile_pool(name="singles", bufs=1))
        feed = ctx.enter_context(tc.tile_pool(name="feed", bufs=3))
        vnp = ctx.enter_context(tc.tile_pool(name="vnp", bufs=3))
        sqp = ctx.enter_context(tc.tile_pool(name="sqp", bufs=2))
        outp = ctx.enter_context(tc.tile_pool(name="outp", bufs=2))
        ps_lg = ctx.enter_context(tc.tile_pool(name="ps_lg", bufs=1, space="PSUM"))
        ps_tr = ctx.enter_context(tc.tile_pool(name="ps_tr", bufs=2, space="PSUM"))
        ps_mm = ctx.enter_context(tc.tile_pool(name="ps_mm", bufs=3, space="PSUM"))

        # --- constants ---
        id_bf = singles.tile([128, 128], bf16)
        make_identity(nc, id_bf)
        id_f32 = singles.tile([128, 128], fp32)
        make_identity(nc, id_f32)
        ones_f32 = singles.tile([1, 128], fp32)
        nc.vector.memset(ones_f32[:], 1.0)

        vn_bf = singles.tile([128, 2, F], bf16)      # natural v, bf16 (mm2 rhs)
        vv_parts = singles.tile([128, 2, F // FCH], fp32)

        # logits PSUM accumulators, one per 128-row half of U.
        # All split-bf16 terms accumulate into the same 256 columns.
        lg = [
            ps_lg.tile([128, 256], fp32, tag=f"lg{m}", name=f"lg{m}")
            for m in range(2)
        ]

        # --- mm1: -2*COEF * u @ v^T, split-bf16, with interleaved vn stream ---
        n_steps = KT // KCH
        vn_jobs = [(vo, c) for vo in range(2) for c in range(F // FCH)]
        vj = 0
        for s in range(n_steps):
            utc = feed.tile([128, KCH, WU], bf16, tag="utc")
            nc.gpsimd.dma_start(utc[:], ut[:, s * KCH:(s + 1) * KCH, :])
            vtc = feed.tile([128, KCH, WV], bf16, tag="vtc")
            nc.gpsimd.dma_start(vtc[:], vt[:, s * KCH:(s + 1) * KCH, :])
            for k in range(KCH):
                kt = s * KCH + k
                for m in range(2):
                    # u_hi stationary: rhs v_hi then (NTERM>=3) v_lo
                    nc.tensor.matmul(
                        lg[m][:],
                        utc[:, k, m * 128:(m + 1) * 128],
                        vtc[:, k, 0:256],
                        start=(kt == 0),
                        stop=False,
                    )
                    if NTERM >= 3:
                        nc.tensor.matmul(
                            lg[m][:],
                            utc[:, k, m * 128:(m + 1) * 128],
                            vtc[:, k, 256:512],
                            start=False,
                            stop=False,
                        )
                    if NTERM >= 2:
                        nc.tensor.matmul(
                            lg[m][:],
                            utc[:, k, 256 + m * 128:256 + (m + 1) * 128],
                            vtc[:, k, 0:256],
                            start=False,
                            stop=False,
                        )
            # spread the 16 natural-v chunk jobs across the k-loop
            want = ((s + 1) * len(vn_jobs)) // n_steps
            while vj < want:
                vo, c = vn_jobs[vj]
                vj += 1
                vch = vnp.tile([128, FCH], fp32, tag="vch")
                nc.gpsimd.dma_start(vch[:], vn[vo, :, c * FCH:(c + 1) * FCH])
                sq = sqp.tile([128, FCH], fp32, tag="sq")
                # both vch readers on ACT so the reused slot's next DMA
                # carries a single engine wait
                nc.scalar.activation(
                    sq[:], vch[:], AF.Square,
                    accum_out=vv_parts[:, vo, c:c + 1],
                )
                nc.scalar.copy(
                    out=vn_bf[:, vo, c * FCH:(c + 1) * FCH], in_=vch[:]
                )

        # --- vv -> COEF*vv as a [1, 256] row, broadcast into the logits ---
        vv2 = singles.tile([128, 2], fp32)
        nc.vector.reduce_sum(vv2[:], vv_parts[:], axis=X)
        vvT_ps = ps_lg.tile([1, 256], fp32, tag="vvT")
        for vo in range(2):
            nc.tensor.transpose(
                vvT_ps[:, vo * 128:(vo + 1) * 128], vv2[:, vo:vo + 1], id_f32
            )
        cvv = singles.tile([1, 256], fp32)
        nc.scalar.mul(cvv[:], vvT_ps[:], COEF)
        for m in range(2):
            nc.tensor.matmul(
                lg[m][:], ones_f32[:], cvv[:], start=False, stop=True
            )

        # --- softmax over v (free dim), normalization deferred to copyback ---
        esc = singles.tile([128, 2, 256], bf16)      # exp(logit - max)
        nm = singles.tile([128, 2], fp32)            # -max
        ssum = singles.tile([128, 2], fp32)          # sum of exps
        rinv = singles.tile([128, 2], fp32)          # 1/sum
        for m in range(2):
            logits_ap = lg[m][:]
            nc.vector.reduce_max(nm[:, m:m + 1], logits_ap, axis=X, negate=True)
            nc.scalar.activation(
                esc[:, m], logits_ap, AF.Exp,
                bias=nm[:, m:m + 1], scale=1.0,
                accum_out=ssum[:, m:m + 1],
            )
            nc.vector.reciprocal(rinv[:, m:m + 1], ssum[:, m:m + 1])

        # --- transpose score to [v, u] layout for the second matmul ---
        scT = singles.tile([128, 2, 256], bf16)      # [vp, vo, u]
        for vo in range(2):
            for m in range(2):
                trp = ps_tr.tile([128, 128], bf16, tag="tr")
                nc.tensor.transpose(
                    trp[:], esc[:, m, vo * 128:(vo + 1) * 128], id_bf
                )
                nc.vector.tensor_copy(
                    out=scT[:, vo, m * 128:(m + 1) * 128], in_=trp[:]
                )

        # --- mm2: neg = (escore @ v) * (1/sum), streamed out in 1MB groups ---
        n_chunks = (F + NCH - 1) // NCH
        chunk_w = [min(NCH, F - ci * NCH) for ci in range(n_chunks)]
        groups = [
            list(range(i, min(i + NG, n_chunks)))
            for i in range(0, n_chunks, NG)
        ]
        for gi, g in enumerate(groups):
            base = g[0] * NCH
            width = sum(chunk_w[ci] for ci in g)
            ost = outp.tile([128, 2, NG * NCH], out_dt, tag="ost")
            # single producer engine (DVE) per group so the store DMA needs
            # only one sync wait; ACT is busy with the vn square+cast
            use_scalar = False
            for ci in g:
                w = chunk_w[ci]
                for m in range(2):
                    ps2 = ps_mm.tile([128, NCH], fp32, tag="ps2")
                    for vo in range(2):
                        nc.tensor.matmul(
                            ps2[:, :w],
                            scT[:, vo, m * 128:(m + 1) * 128],
                            vn_bf[:, vo, ci * NCH:ci * NCH + w],
                            start=(vo == 0),
                            stop=(vo == 1),
                        )
                    dst = ost[:, m, ci * NCH - base:ci * NCH - base + w]
                    if use_scalar:
                        nc.scalar.mul(dst, ps2[:, :w], rinv[:, m:m + 1])
                    else:
                        nc.vector.tensor_scalar_mul(
                            dst, ps2[:, :w], rinv[:, m:m + 1]
                        )
            for m in range(2):
                nc.gpsimd.dma_start(
                    out[m, :, base:base + width], ost[:, m, :width]
                )

    return nc


def _legalize_waits(nc):
    """Make the scheduled BIR fit this walrus build's per-instruction
    sync-wait limits (1 wait for most instruction types, 2 for Matmult).

    Two steps:
      1. Drop DMASW-semaphore waits from DMACopy instructions.  Every
         DMACopy->DMASW wait in this kernel is a write-after-write/read
         ordering against an earlier DMA on the *same* SWDGE logical queue
         (qPoolDynamic) touching the same SBUF partitions: the per-(queue,
         engine) hardware ring drains in FIFO order, so those orderings hold
         by construction and the semaphore wait is redundant (the same
         per-queue dominance argument as tile's optimize_sems pass).
         Reader-side RAW waits on compute instructions are untouched.
      2. Hoist excess waits onto engine NoOps inserted just before the
         offending instruction: same engine queue, so the waits still
         complete before the instruction issues.  (Pattern borrowed from
         tile.py's drain placeholders.)
    """
    import concourse.mybir as mybir

    exempt = (
        mybir.InstAllEngineBarrier,
        mybir.InstEventSemaphore,
        mybir.InstNoOp,
        mybir.InstHalt,
        mybir.InstBranchHint,
        mybir.InstCompareAndBranch,
        mybir.InstCall,
    )

    def limit_for(ins):
        # Matmult lowers to LDWEIGHTS+MATMUL and the waits ride the
        # LDWEIGHTS, which also has a single wait slot.
        return 1

    for fn in nc.m.functions:
        for blk in fn.blocks:
            new_insts = []
            for ins in blk.instructions:
                si = getattr(ins, "sync_info", None)
                if si is not None and si.on_wait and isinstance(
                    ins, mybir.InstDMACopy
                ):
                    si.on_wait = [
                        w for w in si.on_wait
                        if not (w.ant_name or "").startswith("DMASW")
                    ]
                if (
                    si is not None
                    and si.on_wait
                    and not isinstance(ins, exempt)
                ):
                    lim = limit_for(ins)
                    while len(si.on_wait) > lim:
                        w = si.on_wait.pop(0)
                        new_insts.append(
                            mybir.InstNoOp(
                                name=nc.get_next_instruction_name(),
                                engine=ins.engine,
                                sync_info=mybir.SyncInfo(
                                    on_wait=[w], on_update=[]
                                ),
                                bass_nofuse=True,
                            )
                        )
                new_insts.append(ins)
            blk.instructions = new_insts


def _get_nc():
    key = (NTERM, OUT_BF16)
    with _nc_lock:
        if key not in _nc_cache:
            nc = _build_nc()
            _legalize_waits(nc)
            _nc_cache[key] = nc
        return _nc_cache[key]


def _tile_T(x):
    """[U, F] -> [128, KT, U]: partition-tiled transpose (x.T in k-tiles)."""
    return np.ascontiguousarray(x.reshape(x.shape[0], KT, 128).transpose(2, 1, 0))


def kernel(**inputs):
    from concourse.bass_utils import run_bass_kernel_spmd

    global LAST_RESULTS
    feat_u = np.asarray(inputs["feat_u"], dtype=np.float32)
    feat_v = np.asarray(inputs["feat_v"], dtype=np.float32)
    P = feat_u.shape[0]
    assert P == N_CORES and feat_u.shape == (P, U, F) and feat_v.shape == (P, V, F)

    nc = _get_nc()

    in_maps = []
    for p in range(P):
        us = (-2.0 * COEF) * feat_u[p]                 # [U, F], prescaled
        uh = us.astype(BF16)
        if NTERM >= 2:
            ul = (us - uh.astype(np.float32)).astype(BF16)
            ut_np = np.concatenate([_tile_T(uh), _tile_T(ul)], axis=-1)
        else:
            ut_np = _tile_T(uh)
        v = feat_v[p]
        vh = v.astype(BF16)
        if NTERM >= 3:
            vl = (v - vh.astype(np.float32)).astype(BF16)
            vt_np = np.concatenate([_tile_T(vh), _tile_T(vl)], axis=-1)
        else:
            vt_np = _tile_T(vh)
        vn_np = np.ascontiguousarray(v.reshape(2, 128, F))
        in_maps.append({
            "ut": np.ascontiguousarray(ut_np),
            "vt": np.ascontiguousarray(vt_np),
            "vn": vn_np,
        })

    trace = bool(int(os.environ.get("ATHENA_TRACE", "0")))
    res = run_bass_kernel_spmd(
        nc, in_maps, core_ids=list(range(N_CORES)), trace=trace
    )
    LAST_RESULTS = res

    negs = [
        r["out"].astype(np.float32).reshape(U, F) for r in res.results
    ]
    return np.concatenate(
        [feat_u.reshape(-1, F)] + [np.concatenate(negs, 0)], 0
    )


# revision 18
# speedup vs baseline: 1.1464x; 1.1464x over previous
"""Trainium2 Bass kernel for CycleEnergyDirectAddNoHead (retrieval soft-kNN).

Math per pair p (8 pairs, one per NeuronCore):
    dist  = (||u||^2 + ||v||^2 - 2 u@v^T) * COEF        [U, V]
    score = softmax(dist, axis=v)
    neg   = score @ v                                    [U, F]
    out   = concat([u.reshape(-1, F), neg.reshape(-1, F)], 0)

Device-side simplifications:
  * ||u||^2 is constant along the softmax axis -> dropped (softmax invariant).
  * u is pre-scaled by -2*COEF on the host during dtype conversion, so the
    PE accumulates -2*COEF*uv directly; COEF*vv is added into the same PSUM
    via a K=1 broadcast matmul with a ones vector.
  * The big F-contraction matmul runs in split-bf16 (hi/lo) for near-f32
    accuracy at bf16 PE rates.  Host feeds pre-transposed, partition-tiled
    layouts so every DMA is contiguous.
  * softmax normalization is folded into the PSUM->SBUF copyback of the
    second matmul (per-partition 1/sum scale).
"""

import os
import threading

import numpy as np
import ml_dtypes

BF16 = ml_dtypes.bfloat16
COEF = 0.1
N_CORES = 8
U = 256
V = 256
F = 12544
KT = F // 128          # 98 k-tiles for the F-contraction
NTERM = int(os.environ.get("ATHENA_NTERM", "2"))   # split-bf16 terms in uv matmul
OUT_BF16 = bool(int(os.environ.get("ATHENA_OUT_BF16", "1")))
KCH = 14               # k-tiles per mm1 feed DMA chunk (98 = 7*14)
FCH = 3136             # f-chunk for the natural-v stream (12544 = 4*3136)
NCH = 512              # mm2 output chunk (psum bank width)
NG = 8                 # mm2 chunks per output DMA group

WU = 512 if NTERM >= 2 else 256   # packed width of the u^T feed
WV = 512 if NTERM >= 3 else 256   # packed width of the v^T feed

LAST_RESULTS = None    # BassKernelResults of the most recent run (for test.py)

_nc_cache = {}
_nc_lock = threading.Lock()


def _build_nc():
    from contextlib import ExitStack

    import concourse.bass as bass
    import concourse.mybir as mybir
    import concourse.tile as tile
    from concourse.masks import make_identity

    fp32 = mybir.dt.float32
    bf16 = mybir.dt.bfloat16
    out_dt = bf16 if OUT_BF16 else fp32
    AF = mybir.ActivationFunctionType
    X = mybir.AxisListType.X

    nc = bass.Bass()
    ut = nc.dram_tensor("ut", [128, KT, WU], bf16, kind="ExternalInput")
    vt = nc.dram_tensor("vt", [128, KT, WV], bf16, kind="ExternalInput")
    vn = nc.dram_tensor("vn", [2, 128, F], fp32, kind="ExternalInput")
    out = nc.dram_tensor("out", [2, 128, F], out_dt, kind="ExternalOutput")

    with tile.TileContext(nc) as tc, ExitStack() as ctx:
        singles = ctx.enter_context(tc.tile_pool(name="singles", bufs=1))
        feed = ctx.enter_context(tc.tile_pool(name="feed", bufs=2))
        vnp = ctx.enter_context(tc.tile_pool(name="vnp", bufs=2))
        sqp = ctx.enter_context(tc.tile_pool(name="sqp", bufs=1))
        outp = ctx.enter_context(tc.tile_pool(name="outp", bufs=2))
        ps_lg = ctx.enter_context(tc.tile_pool(name="ps_lg", bufs=1, space="PSUM"))
        ps_tr = ctx.enter_context(tc.tile_pool(name="ps_tr", bufs=2, space="PSUM"))
        ps_mm = ctx.enter_context(tc.tile_pool(name="ps_mm", bufs=3, space="PSUM"))

        # --- constants ---
        id_bf = singles.tile([128, 128], bf16)
        make_identity(nc, id_bf)
        id_f32 = singles.tile([128, 128], fp32)
        make_identity(nc, id_f32)
        ones_f32 = singles.tile([1, 128], fp32)
        nc.vector.memset(ones_f32[:], 1.0)

        vn_bf = singles.tile([128, 2, F], bf16)      # natural v, bf16 (mm2 rhs)
        vv_parts = singles.tile([128, 2, F // FCH], fp32)

        # logits PSUM accumulators, one per 128-row half of U.
        # All split-bf16 terms accumulate into the same 256 columns.
        lg = [
            ps_lg.tile([128, 256], fp32, tag=f"lg{m}", name=f"lg{m}")
            for m in range(2)
        ]

        # --- mm1: -2*COEF * u @ v^T, split-bf16, with interleaved vn stream ---
        n_steps = KT // KCH
        vn_jobs = [(vo, c) for vo in range(2) for c in range(F // FCH)]
        vj = 0
        for s in range(n_steps):
            utc = feed.tile([128, KCH, WU], bf16, tag="utc")
            nc.gpsimd.dma_start(utc[:], ut[:, s * KCH:(s + 1) * KCH, :])
            vtc = feed.tile([128, KCH, WV], bf16, tag="vtc")
            nc.gpsimd.dma_start(vtc[:], vt[:, s * KCH:(s + 1) * KCH, :])
            for k in range(KCH):
                kt = s * KCH + k
                for m in range(2):
                    # u_hi stationary: rhs v_hi then (NTERM>=3) v_lo
                    nc.tensor.matmul(
                        lg[m][:],
                        utc[:, k, m * 128:(m + 1) * 128],
                        vtc[:, k, 0:256],
                        start=(kt == 0),
                        stop=False,
                    )
                    if NTERM >= 3:
                        nc.tensor.matmul(
                            lg[m][:],
                            utc[:, k, m * 128:(m + 1) * 128],
                            vtc[:, k, 256:512],
                            start=False,
                            stop=False,
                        )
                    if NTERM >= 2:
                        nc.tensor.matmul(
                            lg[m][:],
                            utc[:, k, 256 + m * 128:256 + (m + 1) * 128],
                            vtc[:, k, 0:256],
                            start=False,
                            stop=False,
                        )
            # spread the 16 natural-v chunk jobs across the k-loop
            want = ((s + 1) * len(vn_jobs)) // n_steps
            while vj < want:
                vo, c = vn_jobs[vj]
                vj += 1
                vch = vnp.tile([128, FCH], fp32, tag="vch")
                nc.gpsimd.dma_start(vch[:], vn[vo, :, c * FCH:(c + 1) * FCH])
                sq = sqp.tile([128, FCH], fp32, tag="sq")
                # both vch readers on ACT so the reused slot's next DMA
                # carries a single engine wait
                nc.scalar.activation(
                    sq[:], vch[:], AF.Square,
                    accum_out=vv_parts[:, vo, c:c + 1],
                )
                nc.scalar.copy(
                    out=vn_bf[:, vo, c * FCH:(c + 1) * FCH], in_=vch[:]
                )

        # --- vv -> COEF*vv as a [1, 256] row, broadcast into the logits ---
        vv2 = singles.tile([128, 2], fp32)
        nc.vector.reduce_sum(vv2[:], vv_parts[:], axis=X)
        vvT_ps = ps_lg.tile([1, 256], fp32, tag="vvT")
        for vo in range(2):
            nc.tensor.transpose(
                vvT_ps[:, vo * 128:(vo + 1) * 128], vv2[:, vo:vo + 1], id_f32
            )
        cvv = singles.tile([1, 256], fp32)
        nc.scalar.mul(cvv[:], vvT_ps[:], COEF)
        for m in range(2):
            nc.tensor.matmul(
                lg[m][:], ones_f32[:], cvv[:], start=False, stop=True
            )

        # --- softmax over v (free dim), normalization deferred to copyback ---
        esc = singles.tile([128, 2, 256], bf16)      # exp(logit - max)
        nm = singles.tile([128, 2], fp32)            # -max
        ssum = singles.tile([128, 2], fp32)          # sum of exps
        rinv = singles.tile([128, 2], fp32)          # 1/sum
        for m in range(2):
            logits_ap = lg[m][:]
            nc.vector.reduce_max(nm[:, m:m + 1], logits_ap, axis=X, negate=True)
            nc.scalar.activation(
                esc[:, m], logits_ap, AF.Exp,
                bias=nm[:, m:m + 1], scale=1.0,
                accum_out=ssum[:, m:m + 1],
            )
            nc.vector.reciprocal(rinv[:, m:m + 1], ssum[:, m:m + 1])

        # --- transpose score to [v, u] layout for the second matmul ---
        scT = singles.tile([128, 2, 256], bf16)      # [vp, vo, u]
        for vo in range(2):
            for m in range(2):
                trp = ps_tr.tile([128, 128], bf16, tag="tr")
                nc.tensor.transpose(
                    trp[:], esc[:, m, vo * 128:(vo + 1) * 128], id_bf
                )
                nc.vector.tensor_copy(
                    out=scT[:, vo, m * 128:(m + 1) * 128], in_=trp[:]
                )

        # --- mm2: neg = (escore @ v) * (1/sum), streamed out in 1MB groups ---
        n_chunks = (F + NCH - 1) // NCH
        chunk_w = [min(NCH, F - ci * NCH) for ci in range(n_chunks)]
        groups = [
            list(range(i, min(i + NG, n_chunks)))
            for i in range(0, n_chunks, NG)
        ]
        for gi, g in enumerate(groups):
            base = g[0] * NCH
            width = sum(chunk_w[ci] for ci in g)
            ost = outp.tile([128, 2, NG * NCH], out_dt, tag="ost")
            # single producer engine (DVE) per group so the store DMA needs
            # only one sync wait; ACT is busy with the vn square+cast
            use_scalar = False
            for ci in g:
                w = chunk_w[ci]
                for m in range(2):
                    ps2 = ps_mm.tile([128, NCH], fp32, tag="ps2")
                    for vo in range(2):
                        nc.tensor.matmul(
                            ps2[:, :w],
                            scT[:, vo, m * 128:(m + 1) * 128],
                            vn_bf[:, vo, ci * NCH:ci * NCH + w],
                            start=(vo == 0),
                            stop=(vo == 1),
                        )
                    dst = ost[:, m, ci * NCH - base:ci * NCH - base + w]
                    if use_scalar:
                        nc.scalar.mul(dst, ps2[:, :w], rinv[:, m:m + 1])
                    else:
                        nc.vector.tensor_scalar_mul(
                            dst, ps2[:, :w], rinv[:, m:m + 1]
                        )
            for m in range(2):
                nc.gpsimd.dma_start(
                    out[m, :, base:base + width], ost[:, m, :width]
                )

    return nc


def _legalize_waits(nc):
    """Make the scheduled BIR fit this walrus build's per-instruction
    sync-wait limits (1 wait for most instruction types, 2 for Matmult).

    Two steps:
      1. Drop DMASW-semaphore waits from DMACopy instructions.  Every
         DMACopy->DMASW wait in this kernel is a write-after-write/read
         ordering against an earlier DMA on the *same* SWDGE logical queue
         (qPoolDynamic) touching the same SBUF partitions: the per-(queue,
         engine) hardware ring drains in FIFO order, so those orderings hold
         by construction and the semaphore wait is redundant (the same
         per-queue dominance argument as tile's optimize_sems pass).
         Reader-side RAW waits on compute instructions are untouched.
      2. Hoist excess waits onto engine NoOps inserted just before the
         offending instruction: same engine queue, so the waits still
         complete before the instruction issues.  (Pattern borrowed from
         tile.py's drain placeholders.)
    """
    import concourse.mybir as mybir

    exempt = (
        mybir.InstAllEngineBarrier,
        mybir.InstEventSemaphore,
        mybir.InstNoOp,
        mybir.InstHalt,
        mybir.InstBranchHint,
        mybir.InstCompareAndBranch,
        mybir.InstCall,
    )

    def limit_for(ins):
        # Matmult lowers to LDWEIGHTS+MATMUL and the waits ride the
        # LDWEIGHTS, which also has a single wait slot.
        return 1

    for fn in nc.m.functions:
        for blk in fn.blocks:
            new_insts = []
            for ins in blk.instructions:
                si = getattr(ins, "sync_info", None)
                if si is not None and si.on_wait and isinstance(
                    ins, mybir.InstDMACopy
                ):
                    si.on_wait = [
                        w for w in si.on_wait
                        if not (w.ant_name or "").startswith("DMASW")
                    ]
                if (
                    si is not None
                    and si.on_wait
                    and not isinstance(ins, exempt)
                ):
                    lim = limit_for(ins)
                    while len(si.on_wait) > lim:
                        w = si.on_wait.pop(0)
                        new_insts.append(
                            mybir.InstNoOp(
                                name=nc.get_next_instruction_name(),
                                engine=ins.engine,
                                sync_info=mybir.SyncInfo(
                                    on_wait=[w], on_update=[]
                                ),
                                bass_nofuse=True,
                            )
                        )
                new_insts.append(ins)
            blk.instructions = new_insts


def _get_nc():
    key = (NTERM, OUT_BF16)
    with _nc_lock:
        if key not in _nc_cache:
            nc = _build_nc()
            _legalize_waits(nc)
            _nc_cache[key] = nc
        return _nc_cache[key]


def _tile_T(x):
    """[U, F] -> [128, KT, U]: partition-tiled transpose (x.T in k-tiles)."""
    return np.ascontiguousarray(x.reshape(x.shape[0], KT, 128).transpose(2, 1, 0))


def kernel(**inputs):
    from concourse.bass_utils import run_bass_kernel_spmd

    global LAST_RESULTS
    feat_u = np.asarray(inputs["feat_u"], dtype=np.float32)
    feat_v = np.asarray(inputs["feat_v"], dtype=np.float32)
    P = feat_u.shape[0]
    assert P == N_CORES and feat_u.shape == (P, U, F) and feat_v.shape == (P, V, F)

    nc = _get_nc()

    in_maps = []
    for p in range(P):
        us = (-2.0 * COEF) * feat_u[p]                 # [U, F], prescaled
        uh = us.astype(BF16)
        if NTERM >= 2:
            ul = (us - uh.astype(np.float32)).astype(BF16)
            ut_np = np.concatenate([_tile_T(uh), _tile_T(ul)], axis=-1)
        else:
            ut_np = _tile_T(uh)
        v = feat_v[p]
        vh = v.astype(BF16)
        if NTERM >= 3:
            vl = (v - vh.astype(np.float32)).astype(BF16)
            vt_np = np.concatenate([_tile_T(vh), _tile_T(vl)], axis=-1)
        else:
            vt_np = _tile_T(vh)
        vn_np = np.ascontiguousarray(v.reshape(2, 128, F))
        in_maps.append({
            "ut": np.ascontiguousarray(ut_np),
            "vt": np.ascontiguousarray(vt_np),
            "vn": vn_np,
        })

    trace = bool(int(os.environ.get("ATHENA_TRACE", "0")))
    res = run_bass_kernel_spmd(
        nc, in_maps, core_ids=list(range(N_CORES)), trace=trace
    )
    LAST_RESULTS = res

    negs = [
        r["out"].astype(np.float32).reshape(U, F) for r in res.results
    ]
    return np.concatenate(
        [feat_u.reshape(-1, F)] + [np.concatenate(negs, 0)], 0
    )
